# revision 10
# baseline (speedup 1.0000x reference)
# Grouped-GEMM "patch readout" kernel for Trainium2 (8 NeuronCores).
#
# Problem: out[b, p, :] = x[b, :, p, :].reshape(T*F) @ W[p] + bias[p]
#   x: [B=32, T=12, P=128, F=128] f32
#   W: [P=128, T*F=1536, NODES*H=768] f32   (604 MB -> the memory-bound term)
#   b: [P=128, 768] f32
#   patch_node_map: [128, 64] int  (permutation; scatter handled on host as the
#   unshard step)
#
# Sharding: expert-parallel over patches. Each of the 8 cores owns 16 patches
# and streams its 75.5 MB W slice from HBM exactly once (system-wide minimum
# traffic). Patches are processed in groups of 4, col-tiled onto the four
# 32-wide column strips of the PE array (output partitions 0/32/64/96) so the
# four M=32 matmuls of a K-chunk row run concurrently. W is streamed as
# one-K-chunk tiles [128, 768] alternating across the two HWDGE rings
# (SP + ACT), matching the t-major consumption order, so PE idle between
# chunk rows stays under the ~3.4us HAM window (no re-throttle) while the
# rings keep the HBM stream dense. x/bias/out ride the gpsimd SWDGE path so
# they never stall the W stream. Bias is added during the PSUM->SBUF
# evacuation (host pre-replicates it across the batch dim).

import numpy as np

import concourse.bacc as bacc
import concourse.mybir as mybir
import concourse.tile as tile
from concourse.bass_utils import run_bass_kernel_spmd

NCORES = 8
B = 32            # batch (matmul M)
T = 12            # timesteps == K chunks of 128 (F == 128)
P = 128           # total patches
F = 128           # features == contraction per chunk
PL = P // NCORES  # 16 patches per core
N = 768           # nodes_per_patch * horizon
NODES_PER_PATCH = 64
HORIZON = 12
N_NODES = P * NODES_PER_PATCH

GRP = 4           # patches per col-tiled group
NGRP = PL // GRP  # 4 groups per core

F32 = mybir.dt.float32

_CACHE = {}


def _build_bass():
    nc = bacc.Bacc("TRN2", target_bir_lowering=False, debug=False)

    # Host-prepared layouts (see kernel()):
    #   xt   [128, PL*T*B]: xt[f, (p*T + t)*B + b] = x[b, t, p_global, f]
    #   w    [PL, T*F, N] : natural per-core W slice
    #   biasr[PL*B, N]    : bias replicated across batch, patch-major
    xt = nc.dram_tensor("xt", [F, PL * T * B], F32, kind="ExternalInput").ap()
    w = nc.dram_tensor("w", [PL, T * F, N], F32, kind="ExternalInput").ap()
    biasr = nc.dram_tensor("biasr", [PL * B, N], F32, kind="ExternalInput").ap()
    out = nc.dram_tensor("out", [PL * B, N], F32, kind="ExternalOutput").ap()

    # [PL, 128(part), T, N] view: chunk (p, t) is W[p, t*128:(t+1)*128, :]
    w4 = w.rearrange("p (t q) n -> p q t n", q=F)

    with tile.TileContext(nc) as tc:
        with (
            tc.tile_pool(name="xpool", bufs=1) as xpool,
            tc.tile_pool(name="wpool", bufs=36) as wpool,
            tc.tile_pool(name="bpool", bufs=2) as bpool,
            tc.tile_pool(name="opool", bufs=2) as opool,
            tc.tile_pool(name="ps", bufs=2, space="PSUM") as pspool,
        ):
            # x lands per group so group 0's slice doesn't wait on the rest
            x_sb = xpool.tile([F, PL * T * B], F32)
            xg = T * B * GRP
            for g in range(NGRP):
                nc.gpsimd.dma_start(
                    x_sb[:, g * xg : (g + 1) * xg], xt[:, g * xg : (g + 1) * xg]
                )

            rings = (nc.sync, nc.scalar)
            w_idx = 0
            for g in range(NGRP):
                bias_sb = bpool.tile([GRP * B, N], F32)
                nc.gpsimd.dma_start(
                    bias_sb[:], biasr[g * GRP * B : (g + 1) * GRP * B]
                )

                ps = pspool.tile([GRP * B, N], F32)
                for t in range(T):
                    for j in range(GRP):
                        p = g * GRP + j
                        wt = wpool.tile([F, N], F32, tag="w")
                        # ACT's ring stalls behind its table-load preamble at
                        # kernel start; keep the first tiles on the SP ring
                        ring = rings[0] if w_idx < 8 else rings[w_idx % 2]
                        ring.dma_start(wt[:], w4[p, :, t])
                        w_idx += 1
                        lhsT = x_sb[:, (p * T + t) * B : (p * T + t + 1) * B]
                        for n0, n1 in ((0, 512), (512, N)):
                            # out partition offset 32*j => col strip j
                            nc.tensor.matmul(
                                ps[j * B : (j + 1) * B, n0:n1],
                                lhsT,
                                wt[:, n0:n1],
                                start=(t == 0),
                                stop=(t == T - 1),
                                tile_position=(0, j * B),
                            )

                o_sb = opool.tile([GRP * B, N], F32)
                nc.vector.tensor_tensor(
                    out=o_sb[:], in0=ps[:], in1=bias_sb[:], op=mybir.AluOpType.add
                )
                # last store on HWDGE: ~0.6us completion vs ~2us via SWDGE
                oeng = nc.scalar if g == NGRP - 1 else nc.gpsimd
                oeng.dma_start(out[g * GRP * B : (g + 1) * GRP * B], o_sb[:])

    nc.finalize()
    return nc


def _get_nc():
    if "nc" not in _CACHE:
        _CACHE["nc"] = _build_bass()
    return _CACHE["nc"]


def _make_in_maps(x, W, b):
    x = np.asarray(x, dtype=np.float32)
    W = np.asarray(W, dtype=np.float32)
    b = np.asarray(b, dtype=np.float32)
    # [f, p, t, b] so each per-core slice reshapes to the SBUF layout directly
    xt_full = np.ascontiguousarray(np.transpose(x, (3, 2, 1, 0)))
    in_maps = []
    for c in range(NCORES):
        p0 = c * PL
        xt = np.ascontiguousarray(xt_full[:, p0 : p0 + PL]).reshape(F, PL * T * B)
        biasr = np.ascontiguousarray(
            np.broadcast_to(b[p0 : p0 + PL, None, :], (PL, B, N))
        ).reshape(PL * B, N)
        in_maps.append({"xt": xt, "w": W[p0 : p0 + PL], "biasr": biasr})
    return in_maps


def _unshard(results, patch_node_map):
    # results[c]["out"]: [PL*B, N] -> global [B, N_NODES, HORIZON] scatter
    out_pbn = np.concatenate(
        [np.asarray(r["out"]).reshape(PL, B, N) for r in results], axis=0
    )
    src = (
        out_pbn.reshape(P, B, NODES_PER_PATCH, HORIZON)
        .transpose(1, 0, 2, 3)
        .reshape(B, N_NODES, HORIZON)
    )
    idx = np.asarray(patch_node_map).reshape(-1).astype(np.int64)
    out_all = np.empty((B, N_NODES, HORIZON), dtype=np.float32)
    out_all[:, idx, :] = src
    return out_all


def run(x, W, b, patch_node_map, trace=False):
    nc = _get_nc()
    in_maps = _make_in_maps(x, W, b)
    res = run_bass_kernel_spmd(
        nc, in_maps, core_ids=list(range(NCORES)), trace=trace
    )
    out_all = _unshard(res.results, patch_node_map)
    return out_all, res


def kernel(x, W, b, patch_node_map):
    out_all, _ = run(x, W, b, patch_node_map)
    return out_all


# revision 11
# speedup vs baseline: 1.0360x; 1.0360x over previous
# Grouped-GEMM "patch readout" kernel for Trainium2 (8 NeuronCores).
#
# Problem: out[b, p, :] = x[b, :, p, :].reshape(T*F) @ W[p] + bias[p]
#   x: [B=32, T=12, P=128, F=128] f32
#   W: [P=128, T*F=1536, NODES*H=768] f32   (604 MB -> the memory-bound term)
#   b: [P=128, 768] f32
#   patch_node_map: [128, 64] int  (permutation; scatter handled on host as the
#   unshard step)
#
# Sharding: expert-parallel over patches. Each of the 8 cores owns 16 patches
# and streams its 75.5 MB W slice from HBM exactly once (system-wide minimum
# traffic). Patches are processed in groups of 4, col-tiled onto the four
# 32-wide column strips of the PE array (output partitions 0/32/64/96) so the
# four M=32 matmuls of a K-chunk row run concurrently. W is streamed as
# one-K-chunk tiles [128, 768] alternating across the two HWDGE rings
# (SP + ACT), matching the t-major consumption order, so PE idle between
# chunk rows stays under the ~3.4us HAM window (no re-throttle) while the
# rings keep the HBM stream dense. x/bias/out ride the gpsimd SWDGE path so
# they never stall the W stream. Bias is added during the PSUM->SBUF
# evacuation (host pre-replicates it across the batch dim).

import numpy as np

import concourse.bacc as bacc
import concourse.mybir as mybir
import concourse.tile as tile
from concourse.bass_utils import run_bass_kernel_spmd

NCORES = 8
B = 32            # batch (matmul M)
T = 12            # timesteps == K chunks of 128 (F == 128)
P = 128           # total patches
F = 128           # features == contraction per chunk
PL = P // NCORES  # 16 patches per core
N = 768           # nodes_per_patch * horizon
NODES_PER_PATCH = 64
HORIZON = 12
N_NODES = P * NODES_PER_PATCH

GRP = 4           # patches per col-tiled group
NGRP = PL // GRP  # 4 groups per core

F32 = mybir.dt.float32

_CACHE = {}


def _build_bass():
    nc = bacc.Bacc("TRN2", target_bir_lowering=False, debug=False)

    # Host-prepared layouts (see kernel()):
    #   xt   [128, PL*T*B]: xt[f, (p*T + t)*B + b] = x[b, t, p_global, f]
    #   w    [PL, T*F, N] : natural per-core W slice
    #   biasr[PL*B, N]    : bias replicated across batch, patch-major
    xt = nc.dram_tensor("xt", [F, PL * T * B], F32, kind="ExternalInput").ap()
    w = nc.dram_tensor("w", [PL, T * F, N], F32, kind="ExternalInput").ap()
    biasr = nc.dram_tensor("biasr", [PL * B, N], F32, kind="ExternalInput").ap()
    out = nc.dram_tensor("out", [PL * B, N], F32, kind="ExternalOutput").ap()

    # [PL, 128(part), T, N] view: chunk (p, t) is W[p, t*128:(t+1)*128, :]
    w4 = w.rearrange("p (t q) n -> p q t n", q=F)

    with tile.TileContext(nc) as tc:
        with (
            tc.tile_pool(name="xpool", bufs=1) as xpool,
            tc.tile_pool(name="wpool", bufs=28) as wpool,
            tc.tile_pool(name="bpool", bufs=2) as bpool,
            tc.tile_pool(name="opool", bufs=2) as opool,
            tc.tile_pool(name="ps", bufs=2, space="PSUM") as pspool,
        ):
            # x lands per group so group 0's slice doesn't wait on the rest
            x_sb = xpool.tile([F, PL * T * B], F32)
            xg = T * B * GRP
            for g in range(NGRP):
                nc.gpsimd.dma_start(
                    x_sb[:, g * xg : (g + 1) * xg], xt[:, g * xg : (g + 1) * xg]
                )

            rings = (nc.sync, nc.scalar)
            for g in range(NGRP):
                bias_sb = bpool.tile([GRP * B, N], F32)
                nc.gpsimd.dma_start(
                    bias_sb[:], biasr[g * GRP * B : (g + 1) * GRP * B]
                )

                ps = pspool.tile([GRP * B, N], F32)
                for t in range(T):
                    for j in range(GRP):
                        p = g * GRP + j
                        wt = wpool.tile([F, N], F32, tag="w")
                        rings[(t * GRP + j) % 2].dma_start(wt[:], w4[p, :, t])
                        lhsT = x_sb[:, (p * T + t) * B : (p * T + t + 1) * B]
                        for n0, n1 in ((0, 512), (512, N)):
                            # out partition offset 32*j => col strip j
                            nc.tensor.matmul(
                                ps[j * B : (j + 1) * B, n0:n1],
                                lhsT,
                                wt[:, n0:n1],
                                start=(t == 0),
                                stop=(t == T - 1),
                                tile_position=(0, j * B),
                            )

                o_sb = opool.tile([GRP * B, N], F32)
                nc.vector.tensor_tensor(
                    out=o_sb[:], in0=ps[:], in1=bias_sb[:], op=mybir.AluOpType.add
                )
                nc.gpsimd.dma_start(out[g * GRP * B : (g + 1) * GRP * B], o_sb[:])

    nc.finalize()
    return nc


def _get_nc():
    if "nc" not in _CACHE:
        _CACHE["nc"] = _build_bass()
    return _CACHE["nc"]


def _make_in_maps(x, W, b):
    x = np.asarray(x, dtype=np.float32)
    W = np.asarray(W, dtype=np.float32)
    b = np.asarray(b, dtype=np.float32)
    # [f, p, t, b] so each per-core slice reshapes to the SBUF layout directly
    xt_full = np.ascontiguousarray(np.transpose(x, (3, 2, 1, 0)))
    in_maps = []
    for c in range(NCORES):
        p0 = c * PL
        xt = np.ascontiguousarray(xt_full[:, p0 : p0 + PL]).reshape(F, PL * T * B)
        biasr = np.ascontiguousarray(
            np.broadcast_to(b[p0 : p0 + PL, None, :], (PL, B, N))
        ).reshape(PL * B, N)
        in_maps.append({"xt": xt, "w": W[p0 : p0 + PL], "biasr": biasr})
    return in_maps


def _unshard(results, patch_node_map):
    # results[c]["out"]: [PL*B, N] -> global [B, N_NODES, HORIZON] scatter
    out_pbn = np.concatenate(
        [np.asarray(r["out"]).reshape(PL, B, N) for r in results], axis=0
    )
    src = (
        out_pbn.reshape(P, B, NODES_PER_PATCH, HORIZON)
        .transpose(1, 0, 2, 3)
        .reshape(B, N_NODES, HORIZON)
    )
    idx = np.asarray(patch_node_map).reshape(-1).astype(np.int64)
    out_all = np.empty((B, N_NODES, HORIZON), dtype=np.float32)
    out_all[:, idx, :] = src
    return out_all


def run(x, W, b, patch_node_map, trace=False):
    nc = _get_nc()
    in_maps = _make_in_maps(x, W, b)
    res = run_bass_kernel_spmd(
        nc, in_maps, core_ids=list(range(NCORES)), trace=trace
    )
    out_all = _unshard(res.results, patch_node_map)
    return out_all, res


def kernel(x, W, b, patch_node_map):
    out_all, _ = run(x, W, b, patch_node_map)
    return out_all
